# revision 1
# baseline (speedup 1.0000x reference)
"""v4: bf16 hi/lo-split matmuls (error ~2^-17); DVE does the exact fp32
state decay in place; ACT computes signs. Host handles lane permutes.

Per step t (threshold 1.0, decay beta):
  psum1 = x_t @ W1x  (+ ones*(-1/2))  + s1_{t-1} @ (-I/2)     [PE, fp32r]
  mem1  = beta*mem1 + psum1                                   [DVE, fp32]
  s1    = Sign(mem1 - 1)   in {-1,+1}; spk = (s+1)/2          [ACT]
  psum2 = s1 @ (W2/2) + ones2*(sum w2/2 - 1/2) + s2_{t-1} @ (-I/2)
  mem2  = beta*mem2 + psum2        -> output
  s2    = Sign(mem2 - 1)           -> output (host: (s+1)/2)

fp32r note: the PE rounds operands to reduced precision. Here the
precision-critical state path runs on DVE in fp32; matmul inputs are either
exact in fp32r ({-1,+1} signs, -1/2 weights) or x/W (one rounding of the
per-step current injection only).

Lane layout per core: 2 slabs x 21 batch-lanes x NCOLS columns.
  x-tile [85, n]: rows (bl,i), row 84 = ones
  s1/mem1 per slab [106/105, NCOLS]; s2/mem2 pair-packed [126, NCOLS]
  (L2 both slabs at psum base 0 via zero-block lhsT; fp32r rejects col tiling)
"""

import numpy as np
from contextlib import ExitStack
from concurrent.futures import ThreadPoolExecutor

T = 10
NI, NH, NO = 4, 5, 3
BETA = 0.95
THR = 1.0
B_FULL = 1_000_000
NCORES = 8

NBL = 21
NSLAB = 2
NCOLS = 2978
NPB = 1024
BC = NSLAB * NBL * NCOLS  # 125,076
BPAD = BC * NCORES        # 1,000,608

XR = NBL * NI             # 84
M1 = NBL * NH             # 105
M2 = NBL * NO             # 63
M2P = 2 * M2              # 126 (pair)

HALF = 0.5


def set_geometry(ncols, npb):
    global NCOLS, NPB, BC, BPAD
    NCOLS, NPB = ncols, npb
    BC = NSLAB * NBL * NCOLS
    BPAD = BC * NCORES


def bf16_split(a):
    import ml_dtypes
    hi = a.astype(ml_dtypes.bfloat16)
    lo = (a.astype(np.float32) - hi.astype(np.float32)).astype(ml_dtypes.bfloat16)
    return hi, lo


def make_weights(w1, w2):
    w1 = np.asarray(w1, np.float32)
    w2 = np.asarray(w2, np.float32)
    import ml_dtypes
    bf = ml_dtypes.bfloat16
    # W1 [85, 128] hi/lo: rows (bl,i) -> w1[h,i]; ones row -> -1/2 (exact)
    w1f = np.zeros((XR + 1, 128), np.float32)
    for bl in range(NBL):
        for i in range(NI):
            for h in range(NH):
                w1f[4 * bl + i, 5 * bl + h] = w1[h, i]
    w1f[XR, 0:M1] = -HALF
    w1h, w1l = bf16_split(w1f)
    # R1 [105, 128]: -I/2 exact in bf16
    r1 = np.zeros((M1, 128), np.float32)
    r1[:, 0:M1] = -HALF * np.eye(M1)
    r1 = r1.astype(bf)
    # W2s per slab [106, 126] zero-block packed; hi/lo
    w2f = np.zeros((2, M1 + 1, M2P), np.float32)
    ones2w = np.float64(0.0)
    for s in range(2):
        for bl in range(NBL):
            for h in range(NH):
                for o in range(NO):
                    w2f[s, 5 * bl + h, 63 * s + 3 * bl + o] = w2[o, h] / 2.0
        for bl in range(NBL):
            for o in range(NO):
                w2f[s, M1, 63 * s + 3 * bl + o] = (
                    w2[o].astype(np.float64).sum() / 2.0 - HALF
                )
    w2h, w2l = bf16_split(w2f)
    # R2 [126, 126]: -I/2 exact
    r2 = (-HALF * np.eye(M2P)).astype(bf)
    return (w1h, w1l), r1, (w2h, w2l), r2


bass_mult = None
bass_add = None


def _init_ops():
    global bass_mult, bass_add
    import concourse.mybir as mybir
    bass_mult = mybir.AluOpType.mult
    bass_add = mybir.AluOpType.add




def _split_multi_waits(nc):
    """Walrus codegen for compute-engine ISA slots accepts only ONE sync-wait
    command per instruction. Tile sometimes attaches 2+ (e.g. own-engine sem +
    a DMA-completion lane). Hoist the extras onto pure-sync EventSemaphore
    instructions inserted just before, on the same engine queue."""
    import concourse.mybir as mybir

    for f in nc.m.functions:
        for blk in f.blocks:
            out = []
            for ins in blk.instructions:
                si = ins.sync_info
                if (
                    si is not None
                    and len(si.on_wait) > 1
                    and not isinstance(ins, mybir.InstEventSemaphore)
                ):
                    waits = list(si.on_wait)
                    for j, w in enumerate(waits[:-1]):
                        out.append(
                            mybir.InstEventSemaphore(
                                name=f"{ins.name}-ws{j}",
                                engine=ins.engine,
                                ins=[],
                                outs=[],
                                sync_info=mybir.SyncInfo(
                                    on_wait=[w], on_update=[]
                                ),
                            )
                        )
                    ins.sync_info = mybir.SyncInfo(
                        on_wait=[waits[-1]], on_update=list(si.on_update)
                    )
                out.append(ins)
            blk.instructions = out


def build_nc_v4(split_waits=True):
    _init_ops()
    import concourse.bass as bass
    import concourse.mybir as mybir
    from concourse.tile import TileContext

    f32 = mybir.dt.float32
    bf16 = mybir.dt.bfloat16
    Act = mybir.ActivationFunctionType

    groups = []
    c0 = 0
    while c0 < NCOLS:
        n = min(NPB, NCOLS - c0)
        groups.append((c0, n))
        c0 += n

    nc = bass.Bass()
    xh_d = nc.declare_dram_parameter("xh", [T, NSLAB, XR, NCOLS], bf16, isOutput=False)
    xl_d = nc.declare_dram_parameter("xl", [T, NSLAB, XR, NCOLS], bf16, isOutput=False)
    w1h_d = nc.declare_dram_parameter("w1h", [XR + 1, 128], bf16, isOutput=False)
    w1l_d = nc.declare_dram_parameter("w1l", [XR + 1, 128], bf16, isOutput=False)
    r1_d = nc.declare_dram_parameter("r1", [M1, 128], bf16, isOutput=False)
    w2ha_d = nc.declare_dram_parameter("w2ha", [M1 + 1, M2P], bf16, isOutput=False)
    w2hb_d = nc.declare_dram_parameter("w2hb", [M1 + 1, M2P], bf16, isOutput=False)
    w2la_d = nc.declare_dram_parameter("w2la", [M1 + 1, M2P], bf16, isOutput=False)
    w2lb_d = nc.declare_dram_parameter("w2lb", [M1 + 1, M2P], bf16, isOutput=False)
    r2_d = nc.declare_dram_parameter("r2", [M2P, M2P], bf16, isOutput=False)
    ones_d = nc.declare_dram_parameter("ones", [1, NCOLS], bf16, isOutput=False)
    spk_d = nc.declare_dram_parameter("spk2", [T, M2P, NCOLS], f32, isOutput=True)
    mem_d = nc.declare_dram_parameter("mem2", [T, M2P, NCOLS], f32, isOutput=True)

    with ExitStack() as ctx:
        tc = ctx.enter_context(TileContext(nc))
        wp = ctx.enter_context(tc.tile_pool(name="wp", bufs=1))
        st = ctx.enter_context(tc.tile_pool(name="st", bufs=1))
        xp = ctx.enter_context(tc.tile_pool(name="xp", bufs=1))
        ps = ctx.enter_context(tc.tile_pool(name="ps", bufs=2, space="PSUM"))

        negone = wp.tile([128, 1], f32, tag="negone")
        nc.vector.memset(negone[:], -1.0)
        w1h = wp.tile([XR + 1, 128], bf16, tag="w1h")
        w1l = wp.tile([XR + 1, 128], bf16, tag="w1l")
        r1 = wp.tile([M1, 128], bf16, tag="r1")
        w2ha = wp.tile([M1 + 1, M2P], bf16, tag="w2ha")
        w2hb = wp.tile([M1 + 1, M2P], bf16, tag="w2hb")
        w2la = wp.tile([M1 + 1, M2P], bf16, tag="w2la")
        w2lb = wp.tile([M1 + 1, M2P], bf16, tag="w2lb")
        r2 = wp.tile([M2P, M2P], bf16, tag="r2")
        for tl, dr in ((w1h, w1h_d), (w1l, w1l_d), (r1, r1_d),
                       (w2ha, w2ha_d), (w2hb, w2hb_d), (w2la, w2la_d),
                       (w2lb, w2lb_d), (r2, r2_d)):
            nc.sync.dma_start(tl[:], dr[:])

        # persistent state, updated in place; one tile per column-group so
        # the per-group pipelines are independent under Tile's dep tracking
        s1t = [[st.tile([M1 + 1, n], bf16, tag=f"s1_{s}_{gi}",
                        name=f"s1_{s}_{gi}") for gi, (c0, n) in enumerate(groups)]
               for s in range(NSLAB)]
        m1t = [[st.tile([M1, n], f32, tag=f"m1_{s}_{gi}",
                        name=f"m1_{s}_{gi}") for gi, (c0, n) in enumerate(groups)]
               for s in range(NSLAB)]
        s2t = [st.tile([M2P, n], bf16, tag=f"s2t_{gi}", name=f"s2t_{gi}")
               for gi, (c0, n) in enumerate(groups)]
        m2t = [st.tile([M2P, n], f32, tag=f"m2t_{gi}", name=f"m2t_{gi}")
               for gi, (c0, n) in enumerate(groups)]

        xhs = [[xp.tile([XR + 1, NPB], bf16, tag=f"xh_{s}_{r}", name=f"xh_{s}_{r}")
                for r in range(3)] for s in range(NSLAB)]
        xls = [[xp.tile([XR, NPB], bf16, tag=f"xl_{s}_{r}", name=f"xl_{s}_{r}")
                for r in range(3)] for s in range(NSLAB)]


        # init: spikes "off" <=> sign = -1; membranes 0; ones rows
        for s in range(NSLAB):
            for r in range(3):
                nc.sync.dma_start(xhs[s][r][XR : XR + 1, :], ones_d[:, 0:NPB])
            for gi, (c0, n) in enumerate(groups):
                nc.vector.memset(s1t[s][gi][0:M1, :], -1.0)
                nc.vector.memset(m1t[s][gi][:], 0.0)
                nc.sync.dma_start(s1t[s][gi][M1 : M1 + 1, :], ones_d[:, 0:n])
        for gi, (c0, n) in enumerate(groups):
            nc.vector.memset(s2t[gi][:], -1.0)
            nc.vector.memset(m2t[gi][:], 0.0)

        def mm(out_ap, w_ap, rhs_ap, start, stop):
            n = out_ap.shape[-1]
            o = 0
            while o < n:
                k = min(512, n - o)
                nc.tensor.matmul(
                    out_ap[:, o : o + k], w_ap, rhs_ap[:, o : o + k],
                    start=start, stop=stop,
                )
                o += k

        for t in range(T):
            for gi, (c0, n) in enumerate(groups):
                cs = slice(c0, c0 + n)
                ring = (t * len(groups) + gi) % 3
                xh_ = [xhs[s][ring] for s in range(NSLAB)]
                xl_ = [xls[s][ring] for s in range(NSLAB)]
                for s in range(NSLAB):
                    nc.sync.dma_start(xh_[s][0:XR, 0:n], xh_d[t, s, :, cs])
                    nc.sync.dma_start(xl_[s][0:XR, 0:n], xl_d[t, s, :, cs])
                for s in range(NSLAB):
                    ps1 = ps.tile([128, n], f32, tag="ps1", name=f"ps1_{t}_{gi}_{s}")
                    mm(ps1[:, 0:n], w1h[:], xh_[s][:, 0:n],
                       start=True, stop=False)
                    mm(ps1[:, 0:n], w1l[0:XR, :], xh_[s][0:XR, 0:n],
                       start=False, stop=False)
                    mm(ps1[:, 0:n], w1h[0:XR, :], xl_[s][:, 0:n],
                       start=False, stop=False)
                    mm(ps1[:, 0:n], r1[:], s1t[s][gi][0:M1, 0:n],
                       start=False, stop=True)
                    # mem1 = beta*mem1 + psum1   (in place, exact fp32)
                    nc.vector.scalar_tensor_tensor(
                        m1t[s][gi][:, 0:n], m1t[s][gi][:, 0:n], BETA,
                        ps1[0:M1, 0:n], bass_mult, bass_add,
                    )
                    # s1 = Sign(mem1 - 1)
                    nc.scalar.activation(
                        s1t[s][gi][0:M1, 0:n], m1t[s][gi][:, 0:n],
                        Act.Sign, bias=negone[0:M1, :],
                    )
                # layer 2 (pair at base 0 via zero-block lhsT)
                ps2 = ps.tile([M2P, n], f32, tag="ps2", name=f"ps2_{t}_{gi}")
                mm(ps2[:, 0:n], w2ha[:], s1t[0][gi][:, 0:n], start=True, stop=False)
                mm(ps2[:, 0:n], w2la[:], s1t[0][gi][:, 0:n], start=False, stop=False)
                mm(ps2[:, 0:n], w2hb[:], s1t[1][gi][:, 0:n], start=False, stop=False)
                mm(ps2[:, 0:n], w2lb[:], s1t[1][gi][:, 0:n], start=False, stop=False)
                mm(ps2[:, 0:n], r2[:], s2t[gi][:, 0:n], start=False, stop=True)
                nc.vector.scalar_tensor_tensor(
                    m2t[gi][:, 0:n], m2t[gi][:, 0:n], BETA, ps2[:, 0:n],
                    bass_mult, bass_add,
                )
                nc.scalar.activation(
                    s2t[gi][:, 0:n], m2t[gi][:, 0:n], Act.Sign,
                    bias=negone[0:M2P, :],
                )
                nc.gpsimd.dma_start(spk_d[t, :, cs], s2t[gi][:, 0:n])
                nc.sync.dma_start(mem_d[t, :, cs], m2t[gi][:, 0:n])

    if split_waits:
        _split_multi_waits(nc)
    return nc


def prep_core_x(xpad, c):
    import ml_dtypes
    bf = ml_dtypes.bfloat16
    xc = xpad[:, c * BC : (c + 1) * BC, :].reshape(T, NSLAB, NBL, NCOLS, NI)
    xc = np.ascontiguousarray(xc.transpose(0, 1, 2, 4, 3)).reshape(
        T, NSLAB, XR, NCOLS
    )
    xh = xc.astype(bf)
    xl = (xc - xh.astype(np.float32)).astype(bf)
    return xh, xl


def unpack_outputs(res_c):
    s2 = res_c["spk2"]
    m2 = res_c["mem2"]
    out_s = np.empty((T, BC, NO), np.float32)
    out_m = np.empty((T, BC, NO), np.float32)
    v_s = out_s.reshape(T, NSLAB, NBL, NCOLS, NO)
    v_m = out_m.reshape(T, NSLAB, NBL, NCOLS, NO)
    for s in range(NSLAB):
        rows = slice(63 * s, 63 * s + M2)
        a = s2[:, rows, :].reshape(T, NBL, NO, NCOLS).transpose(0, 1, 3, 2)
        b = m2[:, rows, :].reshape(T, NBL, NO, NCOLS).transpose(0, 1, 3, 2)
        v_s[:, s] = (a + 1.0) * 0.5
        v_m[:, s] = b
    return out_s, out_m


def kernel(**inputs):
    x = np.asarray(inputs["x"], dtype=np.float32)
    w1 = np.asarray(inputs["w1"], dtype=np.float32)
    w2 = np.asarray(inputs["w2"], dtype=np.float32)

    from concourse.bass_utils import run_bass_kernel_spmd

    nc = build_nc_v4()
    (w1h, w1l), r1, (w2h, w2l), r2 = make_weights(w1, w2)

    import ml_dtypes
    xpad = np.zeros((T, BPAD, NI), dtype=np.float32)
    xpad[:, :B_FULL] = x
    with ThreadPoolExecutor(8) as ex:
        xs = list(ex.map(lambda c: prep_core_x(xpad, c), range(NCORES)))
    onesv = np.ones((1, NCOLS), ml_dtypes.bfloat16)
    in_maps = [
        {"xh": xs[c][0], "xl": xs[c][1], "w1h": w1h, "w1l": w1l, "r1": r1,
         "w2ha": w2h[0], "w2hb": w2h[1], "w2la": w2l[0], "w2lb": w2l[1],
         "r2": r2, "ones": onesv}
        for c in range(NCORES)
    ]

    import time as _time
    _t0 = _time.time()
    res = run_bass_kernel_spmd(nc, in_maps, list(range(NCORES))).results
    print(f"[kernel4] device compile+run {_time.time()-_t0:.1f}s", flush=True)

    spk2 = np.empty((T, BPAD, NO), dtype=np.float32)
    mem2 = np.empty((T, BPAD, NO), dtype=np.float32)

    def fill(c):
        s, m = unpack_outputs(res[c])
        spk2[:, c * BC : (c + 1) * BC] = s
        mem2[:, c * BC : (c + 1) * BC] = m

    with ThreadPoolExecutor(8) as ex:
        list(ex.map(fill, range(NCORES)))
    return spk2[:, :B_FULL], mem2[:, :B_FULL]



# revision 2
# speedup vs baseline: 1.7503x; 1.7503x over previous
"""v5: fp16-pipeline SNN kernel for 8 trn2 cores (pure data parallel).

Per step t (threshold 1, decay beta, spike signs in {-1,+1}):
  psum1 = xh @ W1h (+ones*-1/2) + xh @ W1l + xl @ W1h + s1 @ (-I/2)   [PE]
  mem1  = beta*mem1 + psum1                       [DVE fp32, in place]
  s1    = Sign(mem1 - 1)                          [ACT, fp16 out]
  psum2 = s1a @ W2a + s1a @ W2la + s1b @ W2b + s1b @ W2lb + s2 @ (-I/2)
  mem2  = beta*mem2 + psum2                       [DVE fp32, ping-pong]
  s2    = Sign(mem2 - 1)                          [ACT, fp8e4 out]
Outputs per t: spk = s2 (fp8, host maps (s+1)/2), mem2 f32 (exact).

All matmul operands fp16 (fp8 for the s2/r2 reset pass, +-1/-0.5 exact);
x and W are hi/lo fp16 splits, exact to ~2^-22 — total rel err ~2e-4.
The bf16 hi/lo variant of this scheme measured 2^-17-level (v4); fp16
is strictly better at the same pass count.

Layout per core: 2 slabs x 21 batch-lanes x 3072 columns (BC=129,024).
  x DRAM [T, 84, slab*2*3072] fp16: cols = slab*6144 + hilo*3072 + c
  s1/m1 per slab [106/105, 3072]; s2/m2 pair-packed [126, 3072], 2 bufs
  (ping-pong so output DMAs never stall the recurrence).
Engine use: PE 13 passes/step; DVE 9 STT/step + ACT 9 Sign/step are the
co-bottleneck (~9.5us/step); x loads on qSP HWDGE, outputs on qACT HWDGE.
"""

import numpy as np
from contextlib import ExitStack
from concurrent.futures import ThreadPoolExecutor

T = 10
NI, NH, NO = 4, 5, 3
BETA = 0.95
THR = 1.0
B_FULL = 1_000_000
NCORES = 8

NBL = 21
NSLAB = 2
NCOLS = 3072
NPB = 1024          # column group width (psum tile)
BC = NSLAB * NBL * NCOLS   # 129,024
BPAD = BC * NCORES         # 1,032,192

XR = NBL * NI       # 84
M1 = NBL * NH       # 105
M2P = 2 * NBL * NO  # 126

HALF = 0.5

bass_mult = None
bass_add = None


def _init_ops():
    global bass_mult, bass_add
    import concourse.mybir as mybir
    bass_mult = mybir.AluOpType.mult
    bass_add = mybir.AluOpType.add


def f16_split(a):
    hi = a.astype(np.float16)
    lo = (a.astype(np.float32) - hi.astype(np.float32)).astype(np.float16)
    return hi, lo


def make_weights(w1, w2):
    w1 = np.asarray(w1, np.float32)
    w2 = np.asarray(w2, np.float32)
    f16 = np.float16
    # W1 [85, 128]: rows (bl,i) -> w1[h,i]; ones row -> -1/2 (exact)
    w1f = np.zeros((XR + 1, 128), np.float32)
    for bl in range(NBL):
        for i in range(NI):
            for h in range(NH):
                w1f[4 * bl + i, 5 * bl + h] = w1[h, i]
    w1f[XR, 0:M1] = -HALF
    w1h, w1l = f16_split(w1f)
    # R1 [105, 128]: -I/2 exact
    r1 = np.zeros((M1, 128), np.float32)
    r1[:, 0:M1] = -HALF * np.eye(M1)
    r1 = r1.astype(f16)
    # W2 per slab [106, 126] zero-block packed; consts folded in ones row
    w2f = np.zeros((2, M1 + 1, M2P), np.float32)
    for s in range(2):
        for bl in range(NBL):
            for h in range(NH):
                for o in range(NO):
                    w2f[s, 5 * bl + h, 63 * s + 3 * bl + o] = w2[o, h] / 2.0
        for bl in range(NBL):
            for o in range(NO):
                w2f[s, M1, 63 * s + 3 * bl + o] = (
                    w2[o].astype(np.float64).sum() / 2.0 - HALF
                )
    w2h, w2l = f16_split(w2f)
    # R2 [126, 126]: -I/2 (exact in fp8e4m3)
    r2 = -HALF * np.eye(M2P, dtype=np.float32)
    return (w1h, w1l), r1, (w2h, w2l), r2


def _split_multi_waits(nc):
    """Walrus accepts only ONE sync-wait per compute instruction; hoist
    extras onto pure-sync EventSemaphore instructions."""
    import concourse.mybir as mybir

    for f in nc.m.functions:
        for blk in f.blocks:
            out = []
            for ins in blk.instructions:
                si = ins.sync_info
                if (
                    si is not None
                    and len(si.on_wait) > 1
                    and not isinstance(ins, mybir.InstEventSemaphore)
                ):
                    waits = list(si.on_wait)
                    for j, w in enumerate(waits[:-1]):
                        out.append(
                            mybir.InstEventSemaphore(
                                name=f"{ins.name}-ws{j}",
                                engine=ins.engine,
                                ins=[],
                                outs=[],
                                sync_info=mybir.SyncInfo(
                                    on_wait=[w], on_update=[]
                                ),
                            )
                        )
                    ins.sync_info = mybir.SyncInfo(
                        on_wait=[waits[-1]], on_update=list(si.on_update)
                    )
                out.append(ins)
            blk.instructions = out
    return nc


def build_nc(split_waits=True, mm_chunk=512, reps=1):
    _init_ops()
    import concourse.bass as bass
    import concourse.mybir as mybir
    from concourse.tile import TileContext

    f32 = mybir.dt.float32
    f16 = mybir.dt.float16
    f8 = mybir.dt.float8e4
    Act = mybir.ActivationFunctionType

    NG = NCOLS // NPB
    SLABW = 2 * NCOLS
    XW = NSLAB * SLABW

    nc = bass.Bass()
    xd = nc.declare_dram_parameter("xd", [T, XR, XW], f16, isOutput=False)
    w1h_d = nc.declare_dram_parameter("w1h", [XR + 1, 128], f16, isOutput=False)
    w1l_d = nc.declare_dram_parameter("w1l", [XR + 1, 128], f16, isOutput=False)
    r1_d = nc.declare_dram_parameter("r1", [M1, 128], f16, isOutput=False)
    w2ha_d = nc.declare_dram_parameter("w2ha", [M1 + 1, M2P], f16, isOutput=False)
    w2hb_d = nc.declare_dram_parameter("w2hb", [M1 + 1, M2P], f16, isOutput=False)
    w2la_d = nc.declare_dram_parameter("w2la", [M1 + 1, M2P], f16, isOutput=False)
    w2lb_d = nc.declare_dram_parameter("w2lb", [M1 + 1, M2P], f16, isOutput=False)
    r2_d = nc.declare_dram_parameter("r2", [M2P, M2P], f8, isOutput=False)
    ones_d = nc.declare_dram_parameter("ones", [1, NCOLS], f16, isOutput=False)
    spk_d = nc.declare_dram_parameter("spk2", [T, M2P, NCOLS], f8, isOutput=True)
    mem_d = nc.declare_dram_parameter("mem2", [T, M2P, NCOLS], f32, isOutput=True)

    with ExitStack() as ctx:
        tc = ctx.enter_context(TileContext(nc))
        wp = ctx.enter_context(tc.tile_pool(name="wp", bufs=1))
        st = ctx.enter_context(tc.tile_pool(name="st", bufs=1))
        xp = ctx.enter_context(tc.tile_pool(name="xp", bufs=1))
        ps = ctx.enter_context(tc.tile_pool(name="ps", bufs=2, space="PSUM"))

        negone = wp.tile([128, 1], f32, tag="negone")
        nc.vector.memset(negone[:], -1.0)
        w1h = wp.tile([XR + 1, 128], f16, tag="w1h")
        w1l = wp.tile([XR + 1, 128], f16, tag="w1l")
        r1 = wp.tile([M1, 128], f16, tag="r1")
        w2ha = wp.tile([M1 + 1, M2P], f16, tag="w2ha")
        w2hb = wp.tile([M1 + 1, M2P], f16, tag="w2hb")
        w2la = wp.tile([M1 + 1, M2P], f16, tag="w2la")
        w2lb = wp.tile([M1 + 1, M2P], f16, tag="w2lb")
        r2 = wp.tile([M2P, M2P], f8, tag="r2")
        for tl, dr in ((w1h, w1h_d), (w1l, w1l_d), (r1, r1_d),
                       (w2ha, w2ha_d), (w2hb, w2hb_d), (w2la, w2la_d),
                       (w2lb, w2lb_d), (r2, r2_d)):
            nc.sync.dma_start(tl[:], dr[:])

        # x ring: [85, XW] fp16, row 84 = ones (set once per buffer)
        xts = [xp.tile([XR + 1, XW], f16, tag=f"x_{r}", name=f"x_{r}")
               for r in range(3)]
        for r in range(3):
            for s in range(NSLAB):
                nc.sync.dma_start(
                    xts[r][XR: XR + 1, s * SLABW: s * SLABW + NCOLS],
                    ones_d[:, :],
                )

        # persistent state
        s1t = [st.tile([M1 + 1, NCOLS], f16, tag=f"s1_{s}", name=f"s1_{s}")
               for s in range(NSLAB)]
        m1t = [st.tile([M1, NCOLS], f32, tag=f"m1_{s}", name=f"m1_{s}")
               for s in range(NSLAB)]
        s2t = [st.tile([M2P, NCOLS], f8, tag=f"s2_{p}", name=f"s2_{p}")
               for p in range(2)]
        m2t = [st.tile([M2P, NCOLS], f32, tag=f"m2_{p}", name=f"m2_{p}")
               for p in range(2)]

        for s in range(NSLAB):
            nc.gpsimd.memset(s1t[s][0:M1, :], -1.0)
            nc.gpsimd.memset(m1t[s][:], 0.0)
            nc.sync.dma_start(s1t[s][M1: M1 + 1, :], ones_d[:, :])
        for p in range(2):
            nc.gpsimd.memset(s2t[p][:], -1.0)
            nc.gpsimd.memset(m2t[p][:], 0.0)

        def mm(out_ap, w_ap, rhs_ap, start, stop):
            n = out_ap.shape[-1]
            o = 0
            while o < n:
                k = min(mm_chunk, n - o)
                nc.tensor.matmul(
                    out_ap[:, o: o + k], w_ap, rhs_ap[:, o: o + k],
                    start=start, stop=stop,
                )
                o += k

        for rt in range(reps * T):
            t = rt % T
            xt = xts[rt % 3]
            nc.sync.dma_start(xt[0:XR, :], xd[t])
            cur, prv = rt % 2, (rt + 1) % 2
            for g in range(NG):
                c0 = g * NPB
                cs = slice(c0, c0 + NPB)
                for s in range(NSLAB):
                    hb = s * SLABW + c0          # hi block cols
                    lb = s * SLABW + NCOLS + c0  # lo block cols
                    ps1 = ps.tile([128, NPB], f32, tag="ps1",
                                  name=f"ps1_{rt}_{g}_{s}")
                    mm(ps1[:, :], w1h[:], xt[:, hb: hb + NPB],
                       start=True, stop=False)
                    mm(ps1[:, :], w1l[0:XR, :], xt[0:XR, hb: hb + NPB],
                       start=False, stop=False)
                    mm(ps1[:, :], w1h[0:XR, :], xt[0:XR, lb: lb + NPB],
                       start=False, stop=False)
                    mm(ps1[:, :], r1[:], s1t[s][0:M1, cs],
                       start=False, stop=True)
                    nc.vector.scalar_tensor_tensor(
                        m1t[s][:, cs], m1t[s][:, cs], BETA,
                        ps1[0:M1, :], bass_mult, bass_add,
                    )
                    nc.scalar.activation(
                        s1t[s][0:M1, cs], m1t[s][:, cs],
                        Act.Sign, bias=negone[0:M1, :],
                    )
                ps2 = ps.tile([M2P, NPB], f32, tag="ps2", name=f"ps2_{rt}_{g}")
                mm(ps2[:, :], w2ha[:], s1t[0][:, cs], start=True, stop=False)
                mm(ps2[:, :], w2la[:], s1t[0][:, cs], start=False, stop=False)
                mm(ps2[:, :], w2hb[:], s1t[1][:, cs], start=False, stop=False)
                mm(ps2[:, :], w2lb[:], s1t[1][:, cs], start=False, stop=False)
                mm(ps2[:, :], r2[:], s2t[prv][:, cs], start=False, stop=True)
                nc.vector.scalar_tensor_tensor(
                    m2t[cur][:, cs], m2t[prv][:, cs], BETA, ps2[:, :],
                    bass_mult, bass_add,
                )
                nc.scalar.activation(
                    s2t[cur][:, cs], m2t[cur][:, cs], Act.Sign,
                    bias=negone[0:M2P, :],
                )
            nc.scalar.dma_start(spk_d[t], s2t[cur][:, :])
            nc.scalar.dma_start(mem_d[t], m2t[cur][:, :])

    if split_waits:
        _split_multi_waits(nc)
    return nc


def prep_core_x(xpad, c):
    xc = xpad[:, c * BC: (c + 1) * BC, :].reshape(T, NSLAB, NBL, NCOLS, NI)
    xc = np.ascontiguousarray(xc.transpose(0, 2, 4, 1, 3))  # t, bl, i, s, col
    xc = xc.reshape(T, XR, NSLAB, NCOLS)
    xh = xc.astype(np.float16)
    xl = (xc - xh.astype(np.float32)).astype(np.float16)
    out = np.empty((T, XR, NSLAB, 2, NCOLS), np.float16)
    out[:, :, :, 0, :] = xh
    out[:, :, :, 1, :] = xl
    return out.reshape(T, XR, NSLAB * 2 * NCOLS)


def unpack_outputs(res_c):
    s2 = res_c["spk2"].astype(np.float32)   # [T, 126, NCOLS] in {-1,+1}
    m2 = res_c["mem2"].astype(np.float32)
    out_s = np.empty((T, BC, NO), np.float32)
    out_m = np.empty((T, BC, NO), np.float32)
    v_s = out_s.reshape(T, NSLAB, NBL, NCOLS, NO)
    v_m = out_m.reshape(T, NSLAB, NBL, NCOLS, NO)
    for s in range(NSLAB):
        rows = slice(63 * s, 63 * s + 63)
        a = s2[:, rows, :].reshape(T, NBL, NO, NCOLS).transpose(0, 1, 3, 2)
        b = m2[:, rows, :].reshape(T, NBL, NO, NCOLS).transpose(0, 1, 3, 2)
        v_s[:, s] = (a + 1.0) * 0.5
        v_m[:, s] = b
    return out_s, out_m


def make_in_maps(x, w1, w2):
    import concourse.mybir as mybir
    (w1h, w1l), r1, (w2h, w2l), r2 = make_weights(w1, w2)
    r2 = r2.astype(mybir.dt.np(mybir.dt.float8e4))
    xpad = np.zeros((T, BPAD, NI), dtype=np.float32)
    xpad[:, :B_FULL] = np.asarray(x, np.float32)
    with ThreadPoolExecutor(8) as ex:
        xs = list(ex.map(lambda c: prep_core_x(xpad, c), range(NCORES)))
    onesv = np.ones((1, NCOLS), np.float16)
    return [
        {"xd": xs[c], "w1h": w1h, "w1l": w1l, "r1": r1,
         "w2ha": w2h[0], "w2hb": w2h[1], "w2la": w2l[0], "w2lb": w2l[1],
         "r2": r2, "ones": onesv}
        for c in range(NCORES)
    ]


def kernel(**inputs):
    x = np.asarray(inputs["x"], dtype=np.float32)
    w1 = np.asarray(inputs["w1"], dtype=np.float32)
    w2 = np.asarray(inputs["w2"], dtype=np.float32)

    from concourse.bass_utils import run_bass_kernel_spmd

    nc = build_nc()
    in_maps = make_in_maps(x, w1, w2)

    res = run_bass_kernel_spmd(nc, in_maps, list(range(NCORES))).results

    spk2 = np.empty((T, BPAD, NO), dtype=np.float32)
    mem2 = np.empty((T, BPAD, NO), dtype=np.float32)

    def fill(c):
        s, m = unpack_outputs(res[c])
        spk2[:, c * BC: (c + 1) * BC] = s
        mem2[:, c * BC: (c + 1) * BC] = m

    with ThreadPoolExecutor(8) as ex:
        list(ex.map(fill, range(NCORES)))
    return spk2[:, :B_FULL], mem2[:, :B_FULL]
